# revision 1
# baseline (speedup 1.0000x reference)
"""Trainium2 Bass kernel for causal multi-head attention with RoPE.

Problem: B=2, T=2048, C=2048, H=16, D=128.
Sharding over 8 NeuronCores: batch (2) x head-group (4 heads each); the host
sums the 4 per-head-group partials per batch and adds bo' = bo + bv @ Wo.T
(the v-bias commutes through softmax since rows sum to 1).

v2 design notes:
- All matmuls in float32r (fp32 storage, reduced-precision single-pass PE
  matmul, ~4x faster than fp32, measured rel err ~1.5e-4 at K=2048).
- Transposed-everything layout: xT [C,T], qT/kT [D,T], v [T,D], attnT [D,T],
  out [T,C]; every matmul contracts over partitions, no activation transposes.
- Max-free softmax (scores bounded ~6): scores computed PRE-TRANSPOSED as
  ST = K^T-block x Q-group ([T_k=128, T_q=512] tiles), exp'd directly; row
  sums via a ones-vector matmul accumulated in PSUM; normalization applied to
  attnT via partition-broadcast reciprocal multiply. No PE transposes at all.
- Causality at 128-block granularity: blocks above the diagonal skipped,
  diagonal 128x128 sub-block masked additively, partial-width matmuls
  elsewhere on the diagonal.
- RoPE rotate-half via a PE matmul with a +-1 permutation matrix; 1/sqrt(D)
  folded into Wq/bq on the host.
"""

import math
import sys

import numpy as np

for _p in ("/opt/trn_rl_repo", "/root/.axon_site/_ro/trn_rl_repo"):
    if _p not in sys.path:
        sys.path.append(_p)

import concourse.bacc as bacc
import concourse.bass as bass
import concourse.mybir as mybir
import concourse.tile as tile
from contextlib import ExitStack

F32 = mybir.dt.float32
F32R = mybir.dt.float32r
AF = mybir.ActivationFunctionType
ALU = mybir.AluOpType
AX = mybir.AxisListType

B, T, C = 2, 2048, 2048
H, D = 16, 128
THETA = 10000.0
NEG = -1e9

N_CORES = 8
GROUPS = 4          # head groups (other shard axis is batch)
HPC = H // GROUPS   # heads per core
GW = 512            # T-group width (q-group / proj chunk)


def build_core_nc(T_=T, C_=C, hpc=HPC, debug=False):
    KT = C_ // 128          # contraction k-tiles
    QT = T_ // 128          # 128-wide T tiles
    G = T_ // GW            # 512-wide T groups
    PASSES = hpc // 2

    nc = bacc.Bacc(None, target_bir_lowering=False, debug=debug)

    xT = nc.dram_tensor("xT", [C_, T_], F32R, kind="ExternalInput")
    wqT = nc.dram_tensor("wqT", [C_, hpc * 128], F32R, kind="ExternalInput")
    wkT = nc.dram_tensor("wkT", [C_, hpc * 128], F32R, kind="ExternalInput")
    wvT = nc.dram_tensor("wvT", [C_, hpc * 128], F32R, kind="ExternalInput")
    woT = nc.dram_tensor("woT", [hpc * 128, C_], F32R, kind="ExternalInput")
    bq = nc.dram_tensor("bq", [hpc * 128], F32, kind="ExternalInput")
    bk = nc.dram_tensor("bk", [hpc * 128], F32, kind="ExternalInput")
    cosT = nc.dram_tensor("cosT", [128, T_], F32, kind="ExternalInput")
    sinT = nc.dram_tensor("sinT", [128, T_], F32, kind="ExternalInput")
    maskT = nc.dram_tensor("maskT", [128, 128], F32, kind="ExternalInput")
    rt = nc.dram_tensor("rt", [128, 128], F32R, kind="ExternalInput")
    ones = nc.dram_tensor("ones", [128, 1], F32R, kind="ExternalInput")
    out = nc.dram_tensor("out", [T_, C_], F32, kind="ExternalOutput")

    with tile.TileContext(nc) as tc, ExitStack() as top:
        const = top.enter_context(tc.tile_pool(name="const", bufs=1))
        bq_sb = const.tile([128, hpc], F32, name="bq_sb")
        nc.sync.dma_start(bq_sb[:], bq.rearrange("(h d) -> d h", d=128))
        bk_sb = const.tile([128, hpc], F32, name="bk_sb")
        nc.sync.dma_start(bk_sb[:], bk.rearrange("(h d) -> d h", d=128))
        maskT_sb = const.tile([128, 128], F32, name="maskT_sb")
        nc.sync.dma_start(maskT_sb[:], maskT[:, :])
        rt_sb = const.tile([128, 128], F32R, name="rt_sb")
        nc.sync.dma_start(rt_sb[:], rt[:, :])
        ones_sb = const.tile([128, 1], F32R, name="ones_sb")
        nc.sync.dma_start(ones_sb[:], ones[:, :])

        attnp = top.enter_context(tc.tile_pool(name="attnp", bufs=1))
        attnT = attnp.tile([128, hpc, T_], F32R, name="attnT")

        with ExitStack() as ph:
            xp = ph.enter_context(tc.tile_pool(name="xp", bufs=12))
            wp = ph.enter_context(tc.tile_pool(name="wp", bufs=1))
            kv = ph.enter_context(tc.tile_pool(name="kv", bufs=1))
            qp = ph.enter_context(tc.tile_pool(name="qp", bufs=2))
            raw = ph.enter_context(tc.tile_pool(name="raw", bufs=2))
            ptp = ph.enter_context(tc.tile_pool(name="ptp", bufs=3))
            csp = ph.enter_context(tc.tile_pool(name="csp", bufs=1))
            nrm = ph.enter_context(tc.tile_pool(name="nrm", bufs=2))
            smp = ph.enter_context(tc.tile_pool(name="smp", bufs=1))
            acc = ph.enter_context(tc.tile_pool(name="acc", bufs=2, space="PSUM"))
            stp = ph.enter_context(tc.tile_pool(name="stp", bufs=3, space="PSUM"))
            avp = ph.enter_context(tc.tile_pool(name="avp", bufs=1, space="PSUM"))
            onp = ph.enter_context(tc.tile_pool(name="onp", bufs=1, space="PSUM"))

            for p in range(PASSES):
                pcols = slice(p * 256, (p + 1) * 256)
                wq_sb = wp.tile([128, KT, 256], F32R, tag="wq", name=f"wq_{p}")
                nc.sync.dma_start(
                    wq_sb[:], wqT[:, pcols].rearrange("(ko ki) n -> ki ko n", ki=128)
                )
                wk_sb = wp.tile([128, KT, 256], F32R, tag="wk", name=f"wk_{p}")
                nc.sync.dma_start(
                    wk_sb[:], wkT[:, pcols].rearrange("(ko ki) n -> ki ko n", ki=128)
                )
                wv_sb = wp.tile([128, KT, 256], F32R, tag="wv", name=f"wv_{p}")
                nc.sync.dma_start(
                    wv_sb[:], wvT[:, pcols].rearrange("(ko ki) n -> ki ko n", ki=128)
                )
                kT_sb = kv.tile([128, 2, T_], F32R, tag="kT", name=f"kT_{p}")
                v_sb = kv.tile([128, QT, 256], F32R, tag="v", name=f"v_{p}")
                qts = {}

                def proj_chunk(g, p=p, pcols=pcols, wq_sb=wq_sb, wk_sb=wk_sb,
                               wv_sb=wv_sb, kT_sb=kT_sb, v_sb=v_sb, qts=qts):
                    gcols = slice(g * GW, (g + 1) * GW)
                    x_subs = []
                    for kq in range(KT // 2):
                        xs = xp.tile([128, 2, GW], F32R, tag="x",
                                     name=f"x_{p}_{g}_{kq}")
                        nc.sync.dma_start(
                            xs[:],
                            xT[kq * 256 : (kq + 1) * 256, gcols].rearrange(
                                "(ko ki) t -> ki ko t", ki=128
                            ),
                        )
                        x_subs.append(xs)
                    cos_sb = csp.tile([128, GW], F32, tag="cos", name=f"cos_{p}_{g}")
                    nc.sync.dma_start(cos_sb[:], cosT[:, gcols])
                    sin_sb = csp.tile([128, GW], F32, tag="sin", name=f"sin_{p}_{g}")
                    nc.sync.dma_start(sin_sb[:], sinT[:, gcols])

                    qT_sb = qp.tile([128, 2, GW], F32R, tag="qT", name=f"qT_{p}_{g}")
                    qts[g] = qT_sb

                    raws = {}
                    for wsb, bias_sb, is_q in (
                        (wq_sb, bq_sb, True),
                        (wk_sb, bk_sb, False),
                    ):
                        psums = [
                            acc.tile([128, GW], F32, tag="acc",
                                     name=f"pp_{p}_{g}_{is_q}_{hl}")
                            for hl in range(2)
                        ]
                        for kk in range(KT):
                            for hl in range(2):
                                nc.tensor.matmul(
                                    psums[hl][:],
                                    wsb[:, kk, hl * 128 : (hl + 1) * 128],
                                    x_subs[kk // 2][:, kk % 2, :],
                                    start=(kk == 0),
                                    stop=(kk == KT - 1),
                                )
                        for hl in range(2):
                            h = p * 2 + hl
                            q_raw = raw.tile([128, GW], F32R, tag="raw",
                                             name=f"raw_{p}_{g}_{is_q}_{hl}")
                            nc.scalar.activation(
                                q_raw[:], psums[hl][:], AF.Identity,
                                bias=bias_sb[:, h : h + 1],
                            )
                            raws[(is_q, hl)] = q_raw

                    def rope_pair(is_q, p=p, g=g, gcols=gcols, raws=raws,
                                  cos_sb=cos_sb, sin_sb=sin_sb,
                                  qT_sb=qT_sb, kT_sb=kT_sb):
                        for hl in range(2):
                            q_raw = raws[(is_q, hl)]
                            rps = stp.tile([128, GW], F32, tag="st",
                                           name=f"rot_{p}_{g}_{is_q}_{hl}")
                            nc.tensor.matmul(rps[:], rt_sb[:], q_raw[:],
                                             start=True, stop=True)
                            tcos = raw.tile([128, GW], F32, tag="tcos")
                            nc.vector.tensor_tensor(
                                tcos[:], q_raw[:], cos_sb[:], ALU.mult
                            )
                            usin = raw.tile([128, GW], F32, tag="usin")
                            nc.vector.tensor_tensor(
                                usin[:], rps[:], sin_sb[:], ALU.mult
                            )
                            dest = (
                                qT_sb[:, hl, :] if is_q else kT_sb[:, hl, gcols]
                            )
                            nc.gpsimd.tensor_tensor(
                                dest, tcos[:], usin[:], ALU.add
                            )

                    rope_pair(True)

                    # v projection: two T-tiles at a time, N=256
                    for tpair in range(2):
                        vps = [
                            acc.tile([128, 256], F32, tag="acc",
                                     name=f"vp_{p}_{g}_{tpair}_{ti}")
                            for ti in range(2)
                        ]
                        for kk in range(KT):
                            for ti in range(2):
                                tloc = tpair * 2 + ti
                                nc.tensor.matmul(
                                    vps[ti][:],
                                    x_subs[kk // 2][
                                        :, kk % 2, tloc * 128 : (tloc + 1) * 128
                                    ],
                                    wv_sb[:, kk, :],
                                    start=(kk == 0),
                                    stop=(kk == KT - 1),
                                )
                        for ti in range(2):
                            tt = g * 4 + tpair * 2 + ti
                            nc.scalar.copy(v_sb[:, tt, :], vps[ti][:])
                        if tpair == 0:
                            rope_pair(False)

                def attn_group(g, p=p, kT_sb=kT_sb, v_sb=v_sb, qts=qts):
                    qT_sb = qts[g]
                    for hl in range(2):
                        h = p * 2 + hl
                        av = avp.tile([128, GW], F32, tag="av",
                                      name=f"av_{p}_{g}_{hl}")
                        ons = onp.tile([1, GW], F32, tag="on",
                                       name=f"on_{p}_{g}_{hl}")
                        nblocks = 4 * g + 4

                        def emit_st(j, p=p, g=g, hl=hl, kT_sb=kT_sb,
                                    qT_sb=qT_sb):
                            di = j - 4 * g
                            c0 = di * 128 if di >= 0 else 0
                            st = stp.tile([128, GW], F32, tag="st",
                                          name=f"st_{p}_{g}_{hl}_{j}")
                            nc.tensor.matmul(
                                st[:, c0:GW],
                                kT_sb[:, hl, j * 128 : (j + 1) * 128],
                                qT_sb[:, hl, c0:GW],
                                start=True,
                                stop=True,
                            )
                            if di >= 0:
                                nc.vector.tensor_tensor(
                                    st[:, c0 : c0 + 128],
                                    st[:, c0 : c0 + 128],
                                    maskT_sb[:],
                                    ALU.add,
                                )
                            pt = ptp.tile([128, GW], F32R, tag="pt")
                            nc.scalar.activation(
                                pt[:, c0:GW], st[:, c0:GW], AF.Exp
                            )
                            return c0, pt

                        def emit_consume(j, c0, pt, nblocks=nblocks,
                                         hl=hl, av=av, ons=ons, v_sb=v_sb):
                            nc.tensor.matmul(
                                ons[0:1, c0:GW],
                                ones_sb[:],
                                pt[:, c0:GW],
                                start=(j == 0),
                                stop=(j == nblocks - 1),
                            )
                            nc.tensor.matmul(
                                av[:, c0:GW],
                                v_sb[:, j, hl * 128 : (hl + 1) * 128],
                                pt[:, c0:GW],
                                start=(j == 0),
                                stop=(j == nblocks - 1),
                            )

                        pending = []
                        for j in range(nblocks):
                            pending.append((j, *emit_st(j)))
                            if len(pending) > 2:
                                emit_consume(*pending.pop(0))
                        for item in pending:
                            emit_consume(*item)
                        gcols = slice(g * GW, (g + 1) * GW)
                        nc.scalar.copy(attnT[:, h, gcols], av[:])
                        on_sb = smp.tile([1, GW], F32, tag="on_sb")
                        nc.scalar.copy(on_sb[0:1, :], ons[0:1, :])
                        scr = smp.tile([1, GW], F32, tag="scr")
                        ri1 = smp.tile([1, GW], F32, tag="ri1")
                        nc.vector.reciprocal_approx_accurate(
                            ri1[0:1, :], on_sb[0:1, :], scr[0:1, :]
                        )
                        ri = nrm.tile([128, GW], F32, tag="ri")
                        nc.gpsimd.partition_broadcast(ri[:], ri1[0:1, :])
                        nc.vector.tensor_tensor(
                            attnT[:, h, gcols], attnT[:, h, gcols], ri[:],
                            ALU.mult,
                        )

                proj_chunk(0)
                for g in range(G):
                    if g + 1 < G:
                        proj_chunk(g + 1)
                    attn_group(g)

        # output projection: out[t, c] = sum_dloc attnT[dloc, t] * woT[dloc, c]
        with ExitStack() as oph:
            wop = oph.enter_context(tc.tile_pool(name="wop", bufs=1))
            ops = oph.enter_context(tc.tile_pool(name="ops", bufs=4, space="PSUM"))
            outp = oph.enter_context(tc.tile_pool(name="outp", bufs=3))
            wo_sb = wop.tile([128, hpc, C_], F32R, name="wo_sb")
            nc.sync.dma_start(
                wo_sb[:], woT.rearrange("(ho hi) c -> hi ho c", hi=128)
            )
            for tt in range(QT):
                for ncol in range(C_ // 512):
                    op = ops.tile([128, 512], F32, tag="op")
                    for kc in range(hpc):
                        nc.tensor.matmul(
                            op[:],
                            attnT[:, kc, tt * 128 : (tt + 1) * 128],
                            wo_sb[:, kc, ncol * 512 : (ncol + 1) * 512],
                            start=(kc == 0),
                            stop=(kc == hpc - 1),
                        )
                    osb = outp.tile([128, 512], F32, tag="osb")
                    nc.scalar.copy(osb[:], op[:])
                    nc.sync.dma_start(
                        out[tt * 128 : (tt + 1) * 128, ncol * 512 : (ncol + 1) * 512],
                        osb[:],
                    )

    nc.compile()
    return nc


def _rope_tables(T_, theta=THETA):
    inv = 1.0 / (theta ** (np.arange(0, D, 2, dtype=np.float64) / D))
    t = np.arange(T_, dtype=np.float64)
    fr = np.outer(t, inv)
    emb = np.concatenate([fr, fr], axis=1)
    return (
        np.cos(emb).T.astype(np.float32).copy(),
        np.sin(emb).T.astype(np.float32).copy(),
    )


def _maskT():
    tk = np.arange(128)[:, None]
    c = np.arange(128)[None, :]
    return np.where(c >= tk, 0.0, NEG).astype(np.float32)


def _rot_T():
    R = np.zeros((128, 128), dtype=np.float32)
    half = D // 2
    R[np.arange(half), np.arange(half) + half] = -1.0
    R[np.arange(half) + half, np.arange(half)] = 1.0
    return R.T.copy()


def prep_inputs(x, Wq, bq, Wk, bk, Wv, bv, Wo, bo):
    scale = 1.0 / math.sqrt(D)
    cosT, sinT = _rope_tables(T)
    maskT = _maskT()
    rt = _rot_T()
    ones = np.ones((128, 1), dtype=np.float32)
    xT = [np.ascontiguousarray(x[b].T) for b in range(B)]
    in_maps = []
    for c in range(N_CORES):
        b, g = c // GROUPS, c % GROUPS
        rows = slice(g * HPC * D, (g + 1) * HPC * D)
        in_maps.append(
            {
                "xT": xT[b],
                "wqT": np.ascontiguousarray((Wq[rows] * scale).T),
                "wkT": np.ascontiguousarray(Wk[rows].T),
                "wvT": np.ascontiguousarray(Wv[rows].T),
                "woT": np.ascontiguousarray(Wo[:, rows].T),
                "bq": np.ascontiguousarray(bq[rows] * scale),
                "bk": np.ascontiguousarray(bk[rows]),
                "cosT": cosT,
                "sinT": sinT,
                "maskT": maskT,
                "rt": rt,
                "ones": ones,
            }
        )
    bo_eff = (bo + bv @ Wo.T).astype(np.float32)
    return in_maps, bo_eff


_NC_CACHE = {}


def get_nc():
    if "nc" not in _NC_CACHE:
        _NC_CACHE["nc"] = build_core_nc()
    return _NC_CACHE["nc"]


def kernel(x, Wq, bq, Wk, bk, Wv, bv, Wo, bo):
    x = np.asarray(x, dtype=np.float32)
    args = [np.asarray(a, dtype=np.float32) for a in (Wq, bq, Wk, bk, Wv, bv, Wo, bo)]
    in_maps, bo_eff = prep_inputs(x, *args)
    nc = get_nc()

    from concourse.bass_utils import run_bass_kernel_spmd

    res = run_bass_kernel_spmd(nc, in_maps, core_ids=list(range(N_CORES))).results

    out = np.empty((B, T, C), dtype=np.float32)
    for b in range(B):
        acc_ = res[b * GROUPS]["out"].astype(np.float32).copy()
        for g in range(1, GROUPS):
            acc_ += res[b * GROUPS + g]["out"]
        out[b] = acc_ + bo_eff
    return out



# revision 6
# speedup vs baseline: 94.7313x; 94.7313x over previous
"""Trainium2 Bass kernel for causal multi-head attention with RoPE — v4 (bf16).

Problem: B=2, T=2048, C=2048, H=16, D=128.
Sharding over 8 NeuronCores: batch (2) x head-group (4 heads each); the host
sums the 4 per-head-group partials per batch and adds bo' = bo + bv @ Wo.T
(the v-bias commutes through softmax since rows sum to 1).

v4 design notes (over v2):
- All perf-critical matmul operands bf16 (1 cyc/row at ANY output width);
  PSUM stays f32. Halves DMA bytes and SBUF footprint.
- x loaded ONCE per core (single sweep over T-groups covering all 4 heads),
  split into 4 sub-tile DMAs so the first projection matmuls start early.
- O-projection units interleaved between attention heads and into the final
  score-pipeline drain, so cross-engine exp latency never stalls the PE.
- Output DMA'd directly from PSUM (no SBUF staging copy).
- RoPE rotate-half via PE matmul (f32r, 1 cyc/row at width 512); the
  cos/sin combine runs on DVE, final add on DVE (Pool only does the
  reciprocal partition-broadcast).
- Causality at 128-block granularity, max-free softmax, ones-matmul row sums.
"""

import math
import sys

import numpy as np

for _p in ("/opt/trn_rl_repo", "/root/.axon_site/_ro/trn_rl_repo"):
    if _p not in sys.path:
        sys.path.append(_p)

import ml_dtypes

import concourse.bacc as bacc
import concourse.mybir as mybir
import concourse.tile as tile
from contextlib import ExitStack

F32 = mybir.dt.float32
F32R = mybir.dt.float32r
BF16 = mybir.dt.bfloat16
AF = mybir.ActivationFunctionType
ALU = mybir.AluOpType

B, T, C = 2, 2048, 2048
H, D = 16, 128
THETA = 10000.0
NEG = -1e9
BF = ml_dtypes.bfloat16

N_CORES = 8
GROUPS = 4          # head groups (other shard axis is batch)
HPC = H // GROUPS   # heads per core
GW = 512            # T-group width (q-group / proj chunk)
XSPLIT = 4          # x chunk sub-tiles


def build_core_nc(T_=T, C_=C, hpc=HPC, debug=False, repeat=1):
    KT = C_ // 128          # contraction k-tiles
    QT = T_ // 128          # 128-wide T tiles
    G = T_ // GW            # 512-wide T groups
    KSUB = KT // XSPLIT

    nc = bacc.Bacc(None, target_bir_lowering=False, debug=debug)

    xP = nc.dram_tensor("xP", [T_ // GW, 128, C_ // 128, GW], BF16,
                        kind="ExternalInput")
    wq3 = nc.dram_tensor("wq3", [128, KT, hpc * 128], BF16, kind="ExternalInput")
    wk3 = nc.dram_tensor("wk3", [128, KT, hpc * 128], BF16, kind="ExternalInput")
    wv3 = nc.dram_tensor("wv3", [128, KT, hpc * 128], BF16, kind="ExternalInput")
    wo3 = nc.dram_tensor("wo3", [128, hpc, C_], BF16, kind="ExternalInput")
    bq = nc.dram_tensor("bq", [128, hpc], F32, kind="ExternalInput")
    bk = nc.dram_tensor("bk", [128, hpc], F32, kind="ExternalInput")
    cosT = nc.dram_tensor("cosT", [128, T_], F32, kind="ExternalInput")
    sinT = nc.dram_tensor("sinT", [128, T_], F32, kind="ExternalInput")
    maskT = nc.dram_tensor("maskT", [128, 128], F32, kind="ExternalInput")
    rt = nc.dram_tensor("rt", [128, 128], F32R, kind="ExternalInput")
    ones = nc.dram_tensor("ones", [128, 1], BF16, kind="ExternalInput")
    out = nc.dram_tensor("out", [T_, C_], BF16, kind="ExternalOutput")

    with tile.TileContext(nc) as tc, ExitStack() as top:
        const = top.enter_context(tc.tile_pool(name="const", bufs=1))
        bq_sb = const.tile([128, hpc], F32, name="bq_sb")
        nc.sync.dma_start(bq_sb[:], bq[:, :])
        bk_sb = const.tile([128, hpc], F32, name="bk_sb")
        nc.sync.dma_start(bk_sb[:], bk[:, :])
        maskT_sb = const.tile([128, 128], F32, name="maskT_sb")
        nc.sync.dma_start(maskT_sb[:], maskT[:, :])
        rt_sb = const.tile([128, 128], F32R, name="rt_sb")
        nc.sync.dma_start(rt_sb[:], rt[:, :])
        ones_sb = const.tile([128, 1], BF16, name="ones_sb")
        nc.sync.dma_start(ones_sb[:], ones[:, :])

        pers = top.enter_context(tc.tile_pool(name="pers", bufs=1))
        kT_sb = pers.tile([128, hpc, T_], BF16, name="kT_sb")
        v_sb = pers.tile([128, QT, hpc * 128], BF16, name="v_sb")
        attnT = pers.tile([128, hpc, T_], BF16, name="attnT")

        def _rep_body():
            with ExitStack() as ph:
                wp = ph.enter_context(tc.tile_pool(name="wp", bufs=1))
                xp = ph.enter_context(tc.tile_pool(name="xp", bufs=2 * XSPLIT))
                csp = ph.enter_context(tc.tile_pool(name="csp", bufs=2))
                qp = ph.enter_context(tc.tile_pool(name="qp", bufs=2))
                raw = ph.enter_context(tc.tile_pool(name="raw", bufs=2))
                ptp = ph.enter_context(tc.tile_pool(name="ptp", bufs=5))
                nrm = ph.enter_context(tc.tile_pool(name="nrm", bufs=2))
                smp = ph.enter_context(tc.tile_pool(name="smp", bufs=2))
                outp = ph.enter_context(tc.tile_pool(name="outp", bufs=3))
                acc = ph.enter_context(tc.tile_pool(name="acc", bufs=2, space="PSUM"))
                stp = ph.enter_context(tc.tile_pool(name="stp", bufs=3, space="PSUM"))
                avp = ph.enter_context(tc.tile_pool(name="avp", bufs=1, space="PSUM"))
                onp = ph.enter_context(tc.tile_pool(name="onp", bufs=1, space="PSUM"))
                ops = ph.enter_context(tc.tile_pool(name="ops", bufs=1, space="PSUM"))

                wq_sb = wp.tile([128, KT, hpc * 128], BF16, name="wq_sb")
                wk_sb = wp.tile([128, KT, hpc * 128], BF16, name="wk_sb")
                wv_sb = wp.tile([128, KT, hpc * 128], BF16, name="wv_sb")
                wo_sb = wp.tile([128, hpc, C_], BF16, name="wo_sb")

                def load_w(dst, src, lo, hi):
                    nc.sync.dma_start(dst[:, lo:hi, :], src[:, lo:hi, :])

                load_w(wq_sb, wq3, 0, KT // 4)

                qts = {}

                def proj_chunk(g, first=False):
                    gcols = slice(g * GW, (g + 1) * GW)
                    x_subs = []
                    for xs_i in range(XSPLIT):
                        xs = xp.tile([128, KSUB, GW], BF16, tag="x",
                                     name=f"x_{g}_{xs_i}")
                        nc.sync.dma_start(
                            xs[:],
                            xP[g, :, xs_i * KSUB : (xs_i + 1) * KSUB, :],
                        )
                        x_subs.append(xs)
                        if first and xs_i == 0:
                            load_w(wq_sb, wq3, KT // 4, KT // 2)
                        if first and xs_i == 2:
                            load_w(wk_sb, wk3, 0, KT // 4)
                            load_w(wk_sb, wk3, KT // 4, KT // 2)
                    if first:
                        load_w(wq_sb, wq3, KT // 2, KT)
                        load_w(wk_sb, wk3, KT // 2, KT)
                    cos_sb = csp.tile([128, GW], F32, tag="cos", name=f"cos_{g}")
                    nc.sync.dma_start(cos_sb[:], cosT[:, gcols])
                    sin_sb = csp.tile([128, GW], F32, tag="sin", name=f"sin_{g}")
                    nc.sync.dma_start(sin_sb[:], sinT[:, gcols])
                    if first:
                        load_w(wv_sb, wv3, 0, KT)

                    qT_sb = qp.tile([128, hpc, GW], BF16, tag="qT", name=f"qT_{g}")
                    qts[g] = qT_sb

                    for hp in range(hpc // 2):
                        for wsb, bias_sb, is_q in (
                            (wq_sb, bq_sb, True),
                            (wk_sb, bk_sb, False),
                        ):
                            psums = [
                                acc.tile([128, GW], F32, tag="acc",
                                         name=f"pp_{g}_{hp}_{is_q}_{hl}")
                                for hl in range(2)
                            ]
                            for kk in range(KT):
                                for hl in range(2):
                                    h = hp * 2 + hl
                                    nc.tensor.matmul(
                                        psums[hl][:],
                                        wsb[:, kk, h * 128 : (h + 1) * 128],
                                        x_subs[kk // KSUB][:, kk % KSUB, :],
                                        start=(kk == 0),
                                        stop=(kk == KT - 1),
                                    )
                            for hl in range(2):
                                h = hp * 2 + hl
                                q_raw = raw.tile([128, GW], F32R, tag="raw",
                                                 name=f"raw_{g}_{h}_{is_q}")
                                nc.scalar.activation(
                                    q_raw[:], psums[hl][:], AF.Identity,
                                    bias=bias_sb[:, h : h + 1],
                                )
                                rps = stp.tile([128, GW], F32, tag="st",
                                               name=f"rot_{g}_{h}_{is_q}")
                                nc.tensor.matmul(rps[:], rt_sb[:], q_raw[:],
                                                 start=True, stop=True)
                                tcos = raw.tile([128, GW], F32, tag="tcos")
                                nc.vector.tensor_tensor(
                                    tcos[:], q_raw[:], cos_sb[:], ALU.mult
                                )
                                usin = raw.tile([128, GW], F32, tag="usin")
                                nc.vector.tensor_tensor(
                                    usin[:], rps[:], sin_sb[:], ALU.mult
                                )
                                dest = (
                                    qT_sb[:, h, :] if is_q
                                    else kT_sb[:, h, gcols]
                                )
                                nc.vector.tensor_tensor(
                                    dest, tcos[:], usin[:], ALU.add
                                )

                    # v projection: out rows = t-tile, cols = all heads' d
                    for tloc in range(4):
                        tt = g * 4 + tloc
                        vp = acc.tile([128, hpc * 128], F32, tag="acc",
                                      name=f"vp_{g}_{tloc}")
                        for kk in range(KT):
                            nc.tensor.matmul(
                                vp[:],
                                x_subs[kk // KSUB][
                                    :, kk % KSUB, tloc * 128 : (tloc + 1) * 128
                                ],
                                wv_sb[:, kk, :],
                                start=(kk == 0),
                                stop=(kk == KT - 1),
                            )
                        nc.vector.tensor_copy(v_sb[:, tt, :], vp[:])

                def o_proj_unit(g, u, pools=None):
                    # one (t-tile, ncol-pair) unit: 8 matmuls + copy + DMA out
                    tloc, nc2 = divmod(u, 2)
                    tt = g * 4 + tloc
                    for i, ncol in enumerate((2 * nc2, 2 * nc2 + 1)):
                        pool = (ops if pools is None
                                else pools[(2 * u + i) % len(pools)])
                        tag = ("op" if pool is ops
                               else ("av" if pool is avp else "acc"))
                        op = pool.tile([128, 512], F32, tag=tag)
                        for kc in range(hpc):
                            nc.tensor.matmul(
                                op[:],
                                attnT[:, kc, tt * 128 : (tt + 1) * 128],
                                wo_sb[:, kc, ncol * 512 : (ncol + 1) * 512],
                                start=(kc == 0),
                                stop=(kc == hpc - 1),
                            )
                        osb = outp.tile([128, 512], BF16, tag="osb")
                        nc.vector.tensor_copy(osb[:], op[:])
                        nc.sync.dma_start(
                            out[tt * 128 : (tt + 1) * 128,
                                ncol * 512 : (ncol + 1) * 512],
                            osb[:],
                        )

                def attn_group(g, fillers):
                    # fillers: list of zero-arg emitters (o-proj units of g-1)
                    # interleaved so PE never stalls on the exp pipeline drain.
                    if g == 0:
                        nc.sync.dma_start(wo_sb[:], wo3[:, :, :])
                    qT_sb = qts[g]
                    fi = 0
                    for h in range(hpc):
                        av = avp.tile([128, GW], F32, tag="av",
                                      name=f"av_{g}_{h}")
                        ons = onp.tile([1, GW], F32, tag="on",
                                       name=f"on_{g}_{h}")
                        nblocks = 4 * g + 4

                        def emit_st(j, g=g, h=h, qT_sb=qT_sb):
                            di = j - 4 * g
                            c0 = di * 128 if di >= 0 else 0
                            st = stp.tile([128, GW], F32, tag="st",
                                          name=f"st_{g}_{h}_{j}")
                            nc.tensor.matmul(
                                st[:, c0:GW],
                                kT_sb[:, h, j * 128 : (j + 1) * 128],
                                qT_sb[:, h, c0:GW],
                                start=True,
                                stop=True,
                            )
                            if di >= 0:
                                nc.vector.tensor_tensor(
                                    st[:, c0 : c0 + 128],
                                    st[:, c0 : c0 + 128],
                                    maskT_sb[:],
                                    ALU.add,
                                )
                            pt = ptp.tile([128, GW], BF16, tag="pt")
                            nc.scalar.activation(
                                pt[:, c0:GW], st[:, c0:GW], AF.Exp
                            )
                            return c0, pt

                        def emit_consume(j, c0, pt, nblocks=nblocks,
                                         h=h, av=av, ons=ons):
                            nc.tensor.matmul(
                                ons[0:1, c0:GW],
                                ones_sb[:],
                                pt[:, c0:GW],
                                start=(j == 0),
                                stop=(j == nblocks - 1),
                            )
                            nc.tensor.matmul(
                                av[:, c0:GW],
                                v_sb[:, j, h * 128 : (h + 1) * 128],
                                pt[:, c0:GW],
                                start=(j == 0),
                                stop=(j == nblocks - 1),
                            )

                        pending = []
                        for j in range(nblocks):
                            pending.append((j, *emit_st(j)))
                            if len(pending) > 3:
                                emit_consume(*pending.pop(0))
                        # fill the drain with o-proj units of the previous group
                        for item in pending:
                            if fi < len(fillers):
                                fillers[fi]()
                                fi += 1
                            emit_consume(*item)

                        gcols = slice(g * GW, (g + 1) * GW)
                        on_sb = smp.tile([1, GW], F32, tag="on_sb")
                        nc.scalar.copy(on_sb[0:1, :], ons[0:1, :])
                        scr = smp.tile([1, GW], F32, tag="scr")
                        ri1 = smp.tile([1, GW], F32, tag="ri1")
                        nc.vector.reciprocal_approx_accurate(
                            ri1[0:1, :], on_sb[0:1, :], scr[0:1, :]
                        )
                        ri = nrm.tile([128, GW], F32, tag="ri")
                        nc.gpsimd.partition_broadcast(ri[:], ri1[0:1, :])
                        nc.vector.tensor_tensor(
                            attnT[:, h, gcols], av[:], ri[:], ALU.mult
                        )
                    # any fillers not consumed by the drain slots
                    while fi < len(fillers):
                        fillers[fi]()
                        fi += 1

                proj_chunk(0, first=True)
                for g in range(G):
                    if g + 1 < G:
                        proj_chunk(g + 1)
                    fillers = (
                        [lambda u=u, g=g - 1: o_proj_unit(g, u) for u in range(8)]
                        if g > 0 else []
                    )
                    attn_group(g, fillers)
                for u in range(8):
                    o_proj_unit(G - 1, u, pools=(ops, acc, avp))

        for _ in range(repeat):
            _rep_body()

    nc.compile()
    return nc


def _rope_tables(T_, theta=THETA):
    inv = 1.0 / (theta ** (np.arange(0, D, 2, dtype=np.float64) / D))
    t = np.arange(T_, dtype=np.float64)
    fr = np.outer(t, inv)
    emb = np.concatenate([fr, fr], axis=1)
    cosT = np.cos(emb).T.astype(np.float32).copy()
    sinT = np.sin(emb).T.astype(np.float32).copy()
    return cosT, sinT


def _rot_T():
    R = np.zeros((128, 128), dtype=np.float32)
    half = D // 2
    R[np.arange(half), np.arange(half) + half] = -1.0
    R[np.arange(half) + half, np.arange(half)] = 1.0
    return R.T.copy()


def _maskT():
    tk = np.arange(128)[:, None]
    c = np.arange(128)[None, :]
    return np.where(c >= tk, 0.0, NEG).astype(np.float32)


def prep_inputs(x, Wq, bq, Wk, bk, Wv, bv, Wo, bo):
    scale = 1.0 / math.sqrt(D)
    cosT, sinT = _rope_tables(T)
    maskT = _maskT()
    rt = _rot_T()
    ones = np.ones((128, 1), dtype=BF)
    KTg = C // 128
    Gg = T // GW
    xP = [
        np.ascontiguousarray(
            x[b].T.reshape(KTg, 128, Gg, GW).transpose(2, 1, 0, 3)
        ).astype(BF)
        for b in range(B)
    ]
    KT = C // 128

    def w3(wT):  # [C, N] -> [128, KT, N] (ki, ko, n), contiguous
        n = wT.shape[1]
        return np.ascontiguousarray(
            wT.reshape(KT, 128, n).transpose(1, 0, 2)
        ).astype(BF)

    in_maps = []
    for c in range(N_CORES):
        b, g = c // GROUPS, c % GROUPS
        rows = slice(g * HPC * D, (g + 1) * HPC * D)
        woT = Wo[:, rows].T  # [512, C]
        in_maps.append(
            {
                "xP": xP[b],
                "wq3": w3((Wq[rows] * scale).T),
                "wk3": w3(Wk[rows].T),
                "wv3": w3(Wv[rows].T),
                "wo3": np.ascontiguousarray(
                    woT.reshape(HPC, 128, C).transpose(1, 0, 2)
                ).astype(BF),
                "bq": np.ascontiguousarray((bq[rows] * scale).reshape(HPC, 128).T),
                "bk": np.ascontiguousarray(bk[rows].reshape(HPC, 128).T),
                "cosT": cosT,
                "sinT": sinT,
                "maskT": maskT,
                "rt": rt,
                "ones": ones,
            }
        )
    bo_eff = (bo + bv @ Wo.T).astype(np.float32)
    return in_maps, bo_eff


_NC_CACHE = {}


def get_nc():
    if "nc" not in _NC_CACHE:
        _NC_CACHE["nc"] = build_core_nc()
    return _NC_CACHE["nc"]


def kernel(x, Wq, bq, Wk, bk, Wv, bv, Wo, bo):
    x = np.asarray(x, dtype=np.float32)
    args = [np.asarray(a, dtype=np.float32) for a in (Wq, bq, Wk, bk, Wv, bv, Wo, bo)]
    in_maps, bo_eff = prep_inputs(x, *args)
    nc = get_nc()

    from concourse.bass_utils import run_bass_kernel_spmd

    res = run_bass_kernel_spmd(nc, in_maps, core_ids=list(range(N_CORES))).results

    out = np.empty((B, T, C), dtype=np.float32)
    for b in range(B):
        acc_ = np.asarray(res[b * GROUPS]["out"], dtype=np.float32).copy()
        for g in range(1, GROUPS):
            acc_ += np.asarray(res[b * GROUPS + g]["out"], dtype=np.float32)
        out[b] = acc_ + bo_eff
    return out


# revision 10
# speedup vs baseline: 95.9669x; 1.0130x over previous
"""Trainium2 Bass kernel for causal multi-head attention with RoPE — v7 (bf16).

Problem: B=2, T=2048, C=2048, H=16, D=128.
Sharding over 8 NeuronCores: batch (2) x head-group (4 heads each); the host
sums the 4 per-head-group partials per batch and adds bo' = bo + bv @ Wo.T
(the v-bias commutes through softmax since rows sum to 1).

v6 design notes (over v2):
- All perf-critical matmul operands bf16 (1 cyc/row at ANY output width);
  PSUM stays f32. Halves DMA bytes and SBUF footprint; rel err ~6e-3.
- x loaded ONCE per core (single sweep over T-groups covering all 4 heads),
  host-prearranged to [G,128,KT,GW] so chunk DMAs are fully linear; split
  into sub-tile DMAs so the first projection matmuls start early. Weights
  and biases also host-prearranged into their SBUF layouts.
- O-projection (bf16 output) interleaved: units of group g-1 emitted into
  attention group g's score-pipeline drain slots, so cross-engine exp
  latency never stalls the PE; last group's units alternate across three
  PSUM pools to avoid staging-copy serialization.
- RoPE entirely on DVE: rotate-half done with half-partition multiplies
  against a partition-rotated sign-folded sin table (no PE rotate matmul);
  PSUM->SBUF staging copies (v, out) on DVE to keep ACT free for exp.
- Causality at 128-block granularity, max-free softmax, ones-matmul row
  sums accumulated in PSUM, score pipeline depth 4 (ptp bufs=5, stp=3).
"""

import math
import sys

import numpy as np

for _p in ("/opt/trn_rl_repo", "/root/.axon_site/_ro/trn_rl_repo"):
    if _p not in sys.path:
        sys.path.append(_p)

import ml_dtypes

import concourse.bacc as bacc
import concourse.mybir as mybir
import concourse.tile as tile
from contextlib import ExitStack

F32 = mybir.dt.float32
F32R = mybir.dt.float32r
BF16 = mybir.dt.bfloat16
AF = mybir.ActivationFunctionType
ALU = mybir.AluOpType

B, T, C = 2, 2048, 2048
H, D = 16, 128
THETA = 10000.0
NEG = -1e9
BF = ml_dtypes.bfloat16

N_CORES = 8
GROUPS = 4          # head groups (other shard axis is batch)
HPC = H // GROUPS   # heads per core
GW = 512            # T-group width (q-group / proj chunk)
XSPLIT = 4          # x chunk sub-tiles


def build_core_nc(T_=T, C_=C, hpc=HPC, debug=False, repeat=1):
    KT = C_ // 128          # contraction k-tiles
    QT = T_ // 128          # 128-wide T tiles
    G = T_ // GW            # 512-wide T groups
    KSUB = KT // XSPLIT

    nc = bacc.Bacc(None, target_bir_lowering=False, debug=debug)

    xP = nc.dram_tensor("xP", [T_ // GW, 128, C_ // 128, GW], BF16,
                        kind="ExternalInput")
    wq3 = nc.dram_tensor("wq3", [128, KT, hpc * 128], BF16, kind="ExternalInput")
    wk3 = nc.dram_tensor("wk3", [128, KT, hpc * 128], BF16, kind="ExternalInput")
    wv3 = nc.dram_tensor("wv3", [128, KT, hpc * 128], BF16, kind="ExternalInput")
    wo3 = nc.dram_tensor("wo3", [128, hpc, C_], BF16, kind="ExternalInput")
    bq = nc.dram_tensor("bq", [128, hpc], F32, kind="ExternalInput")
    bk = nc.dram_tensor("bk", [128, hpc], F32, kind="ExternalInput")
    cosT = nc.dram_tensor("cosT", [128, T_], F32, kind="ExternalInput")
    sinT = nc.dram_tensor("sinT", [128, T_], F32, kind="ExternalInput")
    maskT = nc.dram_tensor("maskT", [128, 128], F32, kind="ExternalInput")
    ones = nc.dram_tensor("ones", [128, 1], BF16, kind="ExternalInput")
    out = nc.dram_tensor("out", [T_, C_], BF16, kind="ExternalOutput")

    with tile.TileContext(nc) as tc, ExitStack() as top:
        const = top.enter_context(tc.tile_pool(name="const", bufs=1))
        bq_sb = const.tile([128, hpc], F32, name="bq_sb")
        nc.sync.dma_start(bq_sb[:], bq[:, :])
        bk_sb = const.tile([128, hpc], F32, name="bk_sb")
        nc.sync.dma_start(bk_sb[:], bk[:, :])
        maskT_sb = const.tile([128, 128], F32, name="maskT_sb")
        nc.sync.dma_start(maskT_sb[:], maskT[:, :])
        ones_sb = const.tile([128, 1], BF16, name="ones_sb")
        nc.sync.dma_start(ones_sb[:], ones[:, :])

        pers = top.enter_context(tc.tile_pool(name="pers", bufs=1))
        kT_sb = pers.tile([128, hpc, T_], BF16, name="kT_sb")
        v_sb = pers.tile([128, QT, hpc * 128], BF16, name="v_sb")
        attnT = pers.tile([128, hpc, T_], BF16, name="attnT")

        def _rep_body():
            with ExitStack() as ph:
                wp = ph.enter_context(tc.tile_pool(name="wp", bufs=1))
                xp = ph.enter_context(tc.tile_pool(name="xp", bufs=2 * XSPLIT))
                csp = ph.enter_context(tc.tile_pool(name="csp", bufs=2))
                qp = ph.enter_context(tc.tile_pool(name="qp", bufs=2))
                raw = ph.enter_context(tc.tile_pool(name="raw", bufs=2))
                ptp = ph.enter_context(tc.tile_pool(name="ptp", bufs=5))
                nrm = ph.enter_context(tc.tile_pool(name="nrm", bufs=2))
                smp = ph.enter_context(tc.tile_pool(name="smp", bufs=2))
                outp = ph.enter_context(tc.tile_pool(name="outp", bufs=3))
                acc = ph.enter_context(tc.tile_pool(name="acc", bufs=2, space="PSUM"))
                stp = ph.enter_context(tc.tile_pool(name="stp", bufs=3, space="PSUM"))
                avp = ph.enter_context(tc.tile_pool(name="avp", bufs=1, space="PSUM"))
                onp = ph.enter_context(tc.tile_pool(name="onp", bufs=1, space="PSUM"))
                ops = ph.enter_context(tc.tile_pool(name="ops", bufs=1, space="PSUM"))

                wq_sb = wp.tile([128, KT, hpc * 128], BF16, name="wq_sb")
                wk_sb = wp.tile([128, KT, hpc * 128], BF16, name="wk_sb")
                wv_sb = wp.tile([128, KT, hpc * 128], BF16, name="wv_sb")
                wo_sb = wp.tile([128, hpc, C_], BF16, name="wo_sb")

                def load_w(dst, src, lo, hi):
                    nc.sync.dma_start(dst[:, lo:hi, :], src[:, lo:hi, :])

                load_w(wq_sb, wq3, 0, KT // 4)

                qts = {}

                def proj_chunk(g, first=False):
                    gcols = slice(g * GW, (g + 1) * GW)
                    x_subs = []
                    for xs_i in range(XSPLIT):
                        xs = xp.tile([128, KSUB, GW], BF16, tag="x",
                                     name=f"x_{g}_{xs_i}")
                        nc.sync.dma_start(
                            xs[:],
                            xP[g, :, xs_i * KSUB : (xs_i + 1) * KSUB, :],
                        )
                        x_subs.append(xs)
                        if first and xs_i == 0:
                            load_w(wq_sb, wq3, KT // 4, KT // 2)
                        if first and xs_i == 1:
                            load_w(wk_sb, wk3, 0, KT // 4)
                            load_w(wk_sb, wk3, KT // 4, KT // 2)
                    if first:
                        load_w(wq_sb, wq3, KT // 2, KT)
                        load_w(wk_sb, wk3, KT // 2, KT)
                    cos_sb = csp.tile([128, GW], F32, tag="cos", name=f"cos_{g}")
                    nc.sync.dma_start(cos_sb[:], cosT[:, gcols])
                    sin_sb = csp.tile([128, GW], F32, tag="sin", name=f"sin_{g}")
                    nc.sync.dma_start(sin_sb[:], sinT[:, gcols])
                    if first:
                        load_w(wv_sb, wv3, 0, KT)

                    qT_sb = qp.tile([128, hpc, GW], BF16, tag="qT", name=f"qT_{g}")
                    qts[g] = qT_sb

                    for hp in range(hpc // 2):
                        for wsb, bias_sb, is_q in (
                            (wq_sb, bq_sb, True),
                            (wk_sb, bk_sb, False),
                        ):
                            psums = [
                                acc.tile([128, GW], F32, tag="acc",
                                         name=f"pp_{g}_{hp}_{is_q}_{hl}")
                                for hl in range(2)
                            ]
                            for kk in range(KT):
                                for hl in range(2):
                                    h = hp * 2 + hl
                                    nc.tensor.matmul(
                                        psums[hl][:],
                                        wsb[:, kk, h * 128 : (h + 1) * 128],
                                        x_subs[kk // KSUB][:, kk % KSUB, :],
                                        start=(kk == 0),
                                        stop=(kk == KT - 1),
                                    )
                            for hl in range(2):
                                h = hp * 2 + hl
                                q_raw = raw.tile([128, GW], F32R, tag="raw",
                                                 name=f"raw_{g}_{h}_{is_q}")
                                nc.scalar.activation(
                                    q_raw[:], psums[hl][:], AF.Identity,
                                    bias=bias_sb[:, h : h + 1],
                                )
                                tcos = raw.tile([128, GW], F32, tag="tcos")
                                nc.vector.tensor_tensor(
                                    tcos[:], q_raw[:], cos_sb[:], ALU.mult
                                )
                                usin = raw.tile([128, GW], F32, tag="usin")
                                nc.vector.tensor_tensor(
                                    usin[0:64, :], q_raw[64:128, :],
                                    sin_sb[64:128, :], ALU.mult,
                                )
                                nc.vector.tensor_tensor(
                                    usin[64:128, :], q_raw[0:64, :],
                                    sin_sb[0:64, :], ALU.mult,
                                )
                                dest = (
                                    qT_sb[:, h, :] if is_q
                                    else kT_sb[:, h, gcols]
                                )
                                nc.vector.tensor_tensor(
                                    dest, tcos[:], usin[:], ALU.add
                                )

                    # v projection: out rows = t-tile, cols = all heads' d
                    for tloc in range(4):
                        tt = g * 4 + tloc
                        vp = acc.tile([128, hpc * 128], F32, tag="acc",
                                      name=f"vp_{g}_{tloc}")
                        for kk in range(KT):
                            nc.tensor.matmul(
                                vp[:],
                                x_subs[kk // KSUB][
                                    :, kk % KSUB, tloc * 128 : (tloc + 1) * 128
                                ],
                                wv_sb[:, kk, :],
                                start=(kk == 0),
                                stop=(kk == KT - 1),
                            )
                        nc.vector.tensor_copy(v_sb[:, tt, :], vp[:])

                def o_proj_unit(g, u, pools=None):
                    # one (t-tile, ncol-pair) unit: 8 matmuls + copy + DMA out
                    tloc, nc2 = divmod(u, 2)
                    tt = g * 4 + tloc
                    for i, ncol in enumerate((2 * nc2, 2 * nc2 + 1)):
                        pool = (ops if pools is None
                                else pools[(2 * u + i) % len(pools)])
                        tag = ("op" if pool is ops
                               else ("av" if pool is avp else "acc"))
                        op = pool.tile([128, 512], F32, tag=tag)
                        for kc in range(hpc):
                            nc.tensor.matmul(
                                op[:],
                                attnT[:, kc, tt * 128 : (tt + 1) * 128],
                                wo_sb[:, kc, ncol * 512 : (ncol + 1) * 512],
                                start=(kc == 0),
                                stop=(kc == hpc - 1),
                            )
                        osb = outp.tile([128, 512], BF16, tag="osb")
                        nc.vector.tensor_copy(osb[:], op[:])
                        nc.sync.dma_start(
                            out[tt * 128 : (tt + 1) * 128,
                                ncol * 512 : (ncol + 1) * 512],
                            osb[:],
                        )

                def attn_group(g, fillers):
                    # fillers: list of zero-arg emitters (o-proj units of g-1)
                    # interleaved so PE never stalls on the exp pipeline drain.
                    if g == 0:
                        nc.sync.dma_start(wo_sb[:], wo3[:, :, :])
                    qT_sb = qts[g]
                    fi = 0
                    for h in range(hpc):
                        av = avp.tile([128, GW], F32, tag="av",
                                      name=f"av_{g}_{h}")
                        ons = onp.tile([1, GW], F32, tag="on",
                                       name=f"on_{g}_{h}")
                        nblocks = 4 * g + 4

                        def emit_st(j, g=g, h=h, qT_sb=qT_sb):
                            di = j - 4 * g
                            c0 = di * 128 if di >= 0 else 0
                            st = stp.tile([128, GW], F32, tag="st",
                                          name=f"st_{g}_{h}_{j}")
                            nc.tensor.matmul(
                                st[:, c0:GW],
                                kT_sb[:, h, j * 128 : (j + 1) * 128],
                                qT_sb[:, h, c0:GW],
                                start=True,
                                stop=True,
                            )
                            if di >= 0:
                                nc.vector.tensor_tensor(
                                    st[:, c0 : c0 + 128],
                                    st[:, c0 : c0 + 128],
                                    maskT_sb[:],
                                    ALU.add,
                                )
                            pt = ptp.tile([128, GW], BF16, tag="pt")
                            nc.scalar.activation(
                                pt[:, c0:GW], st[:, c0:GW], AF.Exp
                            )
                            return c0, pt

                        def emit_consume(j, c0, pt, nblocks=nblocks,
                                         h=h, av=av, ons=ons):
                            nc.tensor.matmul(
                                ons[0:1, c0:GW],
                                ones_sb[:],
                                pt[:, c0:GW],
                                start=(j == 0),
                                stop=(j == nblocks - 1),
                            )
                            nc.tensor.matmul(
                                av[:, c0:GW],
                                v_sb[:, j, h * 128 : (h + 1) * 128],
                                pt[:, c0:GW],
                                start=(j == 0),
                                stop=(j == nblocks - 1),
                            )

                        pending = []
                        for j in range(nblocks):
                            pending.append((j, *emit_st(j)))
                            if len(pending) > 3:
                                emit_consume(*pending.pop(0))
                        # fill the drain with o-proj units of the previous group
                        for item in pending:
                            if fi < len(fillers):
                                fillers[fi]()
                                fi += 1
                            emit_consume(*item)

                        gcols = slice(g * GW, (g + 1) * GW)
                        on_sb = smp.tile([1, GW], F32, tag="on_sb")
                        nc.scalar.copy(on_sb[0:1, :], ons[0:1, :])
                        scr = smp.tile([1, GW], F32, tag="scr")
                        ri1 = smp.tile([1, GW], F32, tag="ri1")
                        nc.vector.reciprocal_approx_accurate(
                            ri1[0:1, :], on_sb[0:1, :], scr[0:1, :]
                        )
                        ri = nrm.tile([128, GW], F32, tag="ri")
                        nc.gpsimd.partition_broadcast(ri[:], ri1[0:1, :])
                        nc.vector.tensor_tensor(
                            attnT[:, h, gcols], av[:], ri[:], ALU.mult
                        )
                    # any fillers not consumed by the drain slots
                    while fi < len(fillers):
                        fillers[fi]()
                        fi += 1

                proj_chunk(0, first=True)
                for g in range(G):
                    if g + 1 < G:
                        proj_chunk(g + 1)
                    fillers = (
                        [lambda u=u, g=g - 1: o_proj_unit(g, u) for u in range(8)]
                        if g > 0 else []
                    )
                    attn_group(g, fillers)
                for u in range(8):
                    o_proj_unit(G - 1, u, pools=(ops, acc, avp))

        for _ in range(repeat):
            _rep_body()

    nc.compile()
    return nc


def _rope_tables(T_, theta=THETA):
    inv = 1.0 / (theta ** (np.arange(0, D, 2, dtype=np.float64) / D))
    t = np.arange(T_, dtype=np.float64)
    fr = np.outer(t, inv)
    emb = np.concatenate([fr, fr], axis=1)
    cosT = np.cos(emb).T.astype(np.float32).copy()
    sinT = np.sin(emb).T.astype(np.float32)
    # partition-rotated, sign-folded: sinR[0:64] = +sin[64:128],
    # sinR[64:128] = -sin[0:64]; used by the DVE rotate-half multiplies.
    sinR = np.concatenate([sinT[64:128], -sinT[0:64]], axis=0)
    return cosT, np.ascontiguousarray(sinR)


def _maskT():
    tk = np.arange(128)[:, None]
    c = np.arange(128)[None, :]
    return np.where(c >= tk, 0.0, NEG).astype(np.float32)


def prep_inputs(x, Wq, bq, Wk, bk, Wv, bv, Wo, bo):
    scale = 1.0 / math.sqrt(D)
    cosT, sinT = _rope_tables(T)
    maskT = _maskT()
    ones = np.ones((128, 1), dtype=BF)
    KTg = C // 128
    Gg = T // GW
    xP = [
        np.ascontiguousarray(
            x[b].T.reshape(KTg, 128, Gg, GW).transpose(2, 1, 0, 3)
        ).astype(BF)
        for b in range(B)
    ]
    KT = C // 128

    def w3(wT):  # [C, N] -> [128, KT, N] (ki, ko, n), contiguous
        n = wT.shape[1]
        return np.ascontiguousarray(
            wT.reshape(KT, 128, n).transpose(1, 0, 2)
        ).astype(BF)

    in_maps = []
    for c in range(N_CORES):
        b, g = c // GROUPS, c % GROUPS
        rows = slice(g * HPC * D, (g + 1) * HPC * D)
        woT = Wo[:, rows].T  # [512, C]
        in_maps.append(
            {
                "xP": xP[b],
                "wq3": w3((Wq[rows] * scale).T),
                "wk3": w3(Wk[rows].T),
                "wv3": w3(Wv[rows].T),
                "wo3": np.ascontiguousarray(
                    woT.reshape(HPC, 128, C).transpose(1, 0, 2)
                ).astype(BF),
                "bq": np.ascontiguousarray((bq[rows] * scale).reshape(HPC, 128).T),
                "bk": np.ascontiguousarray(bk[rows].reshape(HPC, 128).T),
                "cosT": cosT,
                "sinT": sinT,
                "maskT": maskT,
                "ones": ones,
            }
        )
    bo_eff = (bo + bv @ Wo.T).astype(np.float32)
    return in_maps, bo_eff


_NC_CACHE = {}


def get_nc():
    if "nc" not in _NC_CACHE:
        _NC_CACHE["nc"] = build_core_nc()
    return _NC_CACHE["nc"]


def kernel(x, Wq, bq, Wk, bk, Wv, bv, Wo, bo):
    x = np.asarray(x, dtype=np.float32)
    args = [np.asarray(a, dtype=np.float32) for a in (Wq, bq, Wk, bk, Wv, bv, Wo, bo)]
    in_maps, bo_eff = prep_inputs(x, *args)
    nc = get_nc()

    from concourse.bass_utils import run_bass_kernel_spmd

    res = run_bass_kernel_spmd(nc, in_maps, core_ids=list(range(N_CORES))).results

    out = np.empty((B, T, C), dtype=np.float32)
    for b in range(B):
        acc_ = np.asarray(res[b * GROUPS]["out"], dtype=np.float32).copy()
        for g in range(1, GROUPS):
            acc_ += np.asarray(res[b * GROUPS + g]["out"], dtype=np.float32)
        out[b] = acc_ + bo_eff
    return out
